# revision 1
# baseline (speedup 1.0000x reference)
"""LoRA layer kernel for Trainium2, 8-core data-parallel.

out = x @ W.T + 2.0 * ((x @ B) @ A)
  x: (4, 4096, 4096) f32, W: (4096, 4096), A: (16, 4096), B: (4096, 16)

Strategy: flatten x to (16384, 4096) rows, shard rows across 8 cores
(2048 rows each), replicate W/A/B. Per core a single fused GEMM:
  - x-block stationary (fp32r), W.T streamed as moving operand
  - LoRA: tT = (x @ B).T computed per block (contraction over full K),
    then one extra K=16 matmul per (m_tile, o_chunk) accumulates
    2*(x@B)@A into the same PSUM bank (A pre-scaled by 2 on host).
All device matmuls use float32r: 1 cycle/row at N=512 (same rate as
bf16, ~TF32 precision).
"""

import sys

if "/opt/trn_rl_repo" not in sys.path:
    sys.path.insert(0, "/opt/trn_rl_repo")

import os

import numpy as np

import concourse.bass as bass
import concourse.mybir as mybir
import concourse.tile as tile

N_CORES = 8
D = 4096
RANK = 16
ROWS_TOTAL = 4 * 4096          # 16384
ROWS_PER_CORE = ROWS_TOTAL // N_CORES  # 2048
P = 128
KT = D // P                    # 32 k-tiles
M_BLOCK = 1024                 # rows per x-resident block
N_BLOCKS = ROWS_PER_CORE // M_BLOCK    # 2
MT_PER_BLOCK = M_BLOCK // P    # 8 m-tiles (PSUM banks)
OC = 512                       # o-chunk width (one PSUM bank)
N_OC = D // OC                 # 8
KH = KT // 2                   # k-tiles per x half-tile

F32 = mybir.dt.float32
F32R = mybir.dt.float32r

W_PAIR = os.environ.get("K_WPAIR", "1") == "1"
GP_DMA = os.environ.get("K_GPDMA", "1") == "1"
WARMUP = os.environ.get("K_WARMUP", "1") == "1"


def _dma_gp(nc):
    return nc.gpsimd if GP_DMA else nc.sync


def split_wide_waits(nc, max_waits=1):
    """walrus in this container rejects >1 sync wait per instruction;
    move excess waits onto preceding same-engine NoOps."""
    n_split = 0
    for f in nc.m.functions:
        for bb in f.blocks:
            new_insts = []
            for inst in bb.instructions:
                si = getattr(inst, "sync_info", None)
                if si is not None and si.on_wait and len(si.on_wait) > max_waits:
                    waits = list(si.on_wait)
                    keep = waits[-max_waits:]
                    extra = waits[:-max_waits]
                    for i in range(0, len(extra), max_waits):
                        chunk = extra[i:i + max_waits]
                        nop = mybir.InstNoOp(
                            name=f"{inst.name}_wsplit{i}",
                            sync_info=mybir.SyncInfo(on_wait=chunk, on_update=[]),
                            bass_nofuse=True,
                            engine=inst.engine,
                        )
                        new_insts.append(nop)
                        n_split += 1
                    si.on_wait = keep
                new_insts.append(inst)
            bb.instructions[:] = new_insts
    return n_split


def build_program():
    nc = bass.Bass()
    xt = nc.declare_dram_parameter("xt", [D, ROWS_PER_CORE], F32R, isOutput=False)
    wt = nc.declare_dram_parameter("wt", [D, D], F32R, isOutput=False)
    # bmat pre-arranged on host: [128, KT*RANK], col-block k = rows k*128..+128
    bmat = nc.declare_dram_parameter("bmat", [P, KT * RANK], F32R, isOutput=False)
    a2 = nc.declare_dram_parameter("a2", [RANK, D], F32R, isOutput=False)
    out = nc.declare_dram_parameter("out", [ROWS_PER_CORE, D], F32, isOutput=True)

    with tile.TileContext(nc) as tc:
        with (
            tc.tile_pool(name="xpool_a", bufs=1) as xpool_a,
            tc.tile_pool(name="xpool_b", bufs=1) as xpool_b,
            tc.tile_pool(name="wpool", bufs=6) as wpool,
            tc.tile_pool(name="opool", bufs=4) as opool,
            tc.tile_pool(name="cpool", bufs=1) as cpool,
            tc.tile_pool(name="tpool", bufs=2) as tpool,
            tc.tile_pool(name="ppool", bufs=8, space="PSUM") as ppool,
        ):
            # constants: B (pre-arranged) and A2 — single DMAs on gpsimd queue
            btile = cpool.tile([P, KT * RANK], F32R, tag="bt")
            _dma_gp(nc).dma_start(btile[:], bmat[:])
            atile = cpool.tile([RANK, D], F32R, tag="at")
            _dma_gp(nc).dma_start(atile[:], a2[:])

            # HAM warmup: ~5us of dummy matmuls so the PE clock is at 8/8
            # before real work lands (3.4us busy window un-throttles).
            if WARMUP:
                junk = ppool.tile([RANK, OC], F32, tag="acc", name="junk")
                for i in range(25):
                    nc.tensor.matmul(
                        junk[:],
                        btile[:, :RANK],
                        btile[:, :OC],
                        start=(i == 0),
                        stop=(i == 24),
                    )

            for blk in range(N_BLOCKS):
                r0 = blk * M_BLOCK
                # x block resident: two half tiles (k 0-15, k 16-31)
                xa = xpool_a.tile([P, KH * M_BLOCK], F32R, tag="xa")
                xb = xpool_b.tile([P, KH * M_BLOCK], F32R, tag="xb")

                def xsl(k, c0, cw):
                    t = xa if k < KH else xb
                    kk = k % KH
                    return t[:, kk * M_BLOCK + c0: kk * M_BLOCK + c0 + cw]

                for k in range(KT):
                    eng = (nc.gpsimd if k % 2 == 0 else nc.scalar) if GP_DMA else nc.sync
                    eng.dma_start(
                        xsl(k, 0, M_BLOCK),
                        xt[k * P:(k + 1) * P, r0:r0 + M_BLOCK],
                    )

                # stage A: tT[r, m] = sum_i B[i,r] * x[m,i]  (per block)
                tT = tpool.tile([RANK, M_BLOCK], F32R, tag="tT")
                for h in range(M_BLOCK // OC):
                    pt = ppool.tile([RANK, OC], F32, tag="acc")
                    for k in range(KT):
                        nc.tensor.matmul(
                            pt[:],
                            btile[:, k * RANK:(k + 1) * RANK],
                            xsl(k, h * OC, OC),
                            start=(k == 0),
                            stop=(k == KT - 1),
                        )
                    nc.vector.tensor_copy(tT[:, h * OC:(h + 1) * OC], pt[:])

                # main GEMM + fused LoRA accumulation.
                # W fetched as adjacent k-tile pairs [128, 2*OC] (halves the
                # ~0.6us/DMA issue count on the sync queue).
                for oc in range(N_OC):
                    psums = []
                    for mt in range(MT_PER_BLOCK):
                        psums.append(ppool.tile([P, OC], F32, tag="acc", name=f"ps_{blk}_{oc}_{mt}"))
                    for k2 in range(KT // 2):
                        wtile = wpool.tile([P, 2 * OC], F32R, tag="wt")
                        src = wt[k2 * 2 * P:(k2 + 1) * 2 * P,
                                 oc * OC:(oc + 1) * OC]
                        if W_PAIR:
                            nc.sync.dma_start(
                                wtile.rearrange("p (b c) -> p b c", b=2),
                                src.rearrange("(b p) c -> p b c", p=P),
                            )
                        else:
                            for half in range(2):
                                nc.sync.dma_start(
                                    wtile[:, half * OC:(half + 1) * OC],
                                    wt[(2 * k2 + half) * P:(2 * k2 + half + 1) * P,
                                       oc * OC:(oc + 1) * OC],
                                )
                        for half in range(2):
                            k = 2 * k2 + half
                            for mt in range(MT_PER_BLOCK):
                                nc.tensor.matmul(
                                    psums[mt][:],
                                    xsl(k, mt * P, P),
                                    wtile[:, half * OC:(half + 1) * OC],
                                    start=(k == 0),
                                    stop=False,
                                )
                    for mt in range(MT_PER_BLOCK):
                        # LoRA: += tT[:, mt].T @ (2A[:, oc])
                        nc.tensor.matmul(
                            psums[mt][:],
                            tT[:, mt * P:(mt + 1) * P],
                            atile[:, oc * OC:(oc + 1) * OC],
                            start=False,
                            stop=True,
                        )
                        ot = opool.tile([P, OC], F32, tag="ot")
                        nc.vector.tensor_copy(ot[:], psums[mt][:])
                        nc.sync.dma_start(
                            out[r0 + mt * P:r0 + (mt + 1) * P,
                                oc * OC:(oc + 1) * OC],
                            ot[:],
                        )

    split_wide_waits(nc)
    return nc


_NC_CACHE = [None]


def kernel(x, weight, lora_A, lora_B):
    from concourse.bass_utils import run_bass_kernel_spmd

    x = np.asarray(x, dtype=np.float32)
    weight = np.asarray(weight, dtype=np.float32)
    lora_A = np.asarray(lora_A, dtype=np.float32)
    lora_B = np.asarray(lora_B, dtype=np.float32)

    x2 = x.reshape(ROWS_TOTAL, D)
    wt = np.ascontiguousarray(weight.T)
    a2 = np.ascontiguousarray(2.0 * lora_A)
    # pre-arrange B: [128, KT*RANK], col-block k holds rows k*128..(k+1)*128
    bmat = np.ascontiguousarray(
        lora_B.reshape(KT, P, RANK).transpose(1, 0, 2).reshape(P, KT * RANK)
    )

    in_maps = []
    for c in range(N_CORES):
        xt_c = np.ascontiguousarray(
            x2[c * ROWS_PER_CORE:(c + 1) * ROWS_PER_CORE].T
        )
        in_maps.append({"xt": xt_c, "wt": wt, "bmat": bmat, "a2": a2})

    if _NC_CACHE[0] is None:
        _NC_CACHE[0] = build_program()
    nc = _NC_CACHE[0]

    res = run_bass_kernel_spmd(nc, in_maps, list(range(N_CORES)))
    out = np.concatenate(
        [res.results[c]["out"] for c in range(N_CORES)], axis=0
    )
    return out.reshape(x.shape)



# revision 2
# speedup vs baseline: 1.1716x; 1.1716x over previous
"""LoRA layer kernel for Trainium2, 8-core data-parallel.

out = x @ W.T + 2.0 * ((x @ B) @ A) = x @ (W.T + 2*(B@A)) = x @ Weff

The LoRA path is folded into the weight on the HOST (B@A is a tiny
rank-16 outer product, ~0.5 GFLOP in numpy) so the device kernel is a
single dense GEMM: out[16384, 4096] = x[16384, 4096] @ Weff[4096, 4096].

Sharding: data-parallel over rows, 2048 rows/core, Weff replicated.

Per-core kernel: both operands cast to bf16 on host (tolerance is
rel_err < 2e-2; bf16 gives ~3e-3). Rows split in two resident blocks of
1024 (8 m-tiles of 128); for each block, 8 output chunks of 512 (one
PSUM bank each) accumulate over 32 k-tiles; W streamed as [128, 2*512]
pair tiles. 4096 MMs of [128x128x512] @ ~225ns = ~922us PE floor.
"""

import sys

if "/opt/trn_rl_repo" not in sys.path:
    sys.path.insert(0, "/opt/trn_rl_repo")

import os

import numpy as np
import ml_dtypes

import concourse.bass as bass
import concourse.mybir as mybir
import concourse.tile as tile

N_CORES = 8
D = 4096
ROWS_TOTAL = 4 * 4096          # 16384
ROWS_PER_CORE = ROWS_TOTAL // N_CORES  # 2048
P = 128
KT = D // P                    # 32 k-tiles
M_BLOCK = 1024                 # rows per x-resident block
N_BLOCKS = ROWS_PER_CORE // M_BLOCK    # 2
MT_PER_BLOCK = M_BLOCK // P    # 8 m-tiles (PSUM banks)
OC = 512                       # o-chunk width (one PSUM bank)
N_OC = D // OC                 # 8
KH = KT // 2                   # k-tiles per x half-tile

F32 = mybir.dt.float32
BF16 = mybir.dt.bfloat16

WARMUP = os.environ.get("K_WARMUP", "1") == "1"


def split_wide_waits(nc, max_waits=1):
    """walrus in this container rejects >1 sync wait per instruction;
    move excess waits onto preceding same-engine NoOps."""
    n_split = 0
    for f in nc.m.functions:
        for bb in f.blocks:
            new_insts = []
            for inst in bb.instructions:
                si = getattr(inst, "sync_info", None)
                if si is not None and si.on_wait and len(si.on_wait) > max_waits:
                    waits = list(si.on_wait)
                    keep = waits[-max_waits:]
                    extra = waits[:-max_waits]
                    for i in range(0, len(extra), max_waits):
                        chunk = extra[i:i + max_waits]
                        nop = mybir.InstNoOp(
                            name=f"{inst.name}_wsplit{i}",
                            sync_info=mybir.SyncInfo(on_wait=chunk, on_update=[]),
                            bass_nofuse=True,
                            engine=inst.engine,
                        )
                        new_insts.append(nop)
                        n_split += 1
                    si.on_wait = keep
                new_insts.append(inst)
            bb.instructions[:] = new_insts
    return n_split


def build_program():
    nc = bass.Bass()
    xt = nc.declare_dram_parameter("xt", [D, ROWS_PER_CORE], BF16, isOutput=False)
    wt = nc.declare_dram_parameter("wt", [D, D], BF16, isOutput=False)
    cz = nc.declare_dram_parameter("cz", [P, OC], BF16, isOutput=False)
    out = nc.declare_dram_parameter("out", [ROWS_PER_CORE, D], F32, isOutput=True)

    with tile.TileContext(nc) as tc:
        with (
            tc.tile_pool(name="xpool_a", bufs=2) as xpool_a,
            tc.tile_pool(name="xpool_b", bufs=2) as xpool_b,
            tc.tile_pool(name="wpool", bufs=6) as wpool,
            tc.tile_pool(name="opool", bufs=4) as opool,
            tc.tile_pool(name="cpool", bufs=1) as cpool,
            tc.tile_pool(name="ppool", bufs=8, space="PSUM") as ppool,
        ):
            # zeros tile for HAM warmup matmuls
            ztile = cpool.tile([P, OC], BF16, tag="zt")
            nc.sync.dma_start(ztile[:], cz[:])

            # HAM warmup: ~5us of dummy matmuls so the PE clock is at 8/8
            # before real work lands.
            if WARMUP:
                junk = ppool.tile([P, OC], F32, tag="acc", name="junk")
                for i in range(22):
                    nc.tensor.matmul(
                        junk[:],
                        ztile[:, :P],
                        ztile[:],
                        start=(i == 0),
                        stop=(i == 21),
                    )

            for blk in range(N_BLOCKS):
                r0 = blk * M_BLOCK
                # x block resident: two half tiles (k 0-15, k 16-31)
                xa = xpool_a.tile([P, KH * M_BLOCK], BF16, tag="xa")
                xb = xpool_b.tile([P, KH * M_BLOCK], BF16, tag="xb")

                def xsl(k, c0, cw):
                    t = xa if k < KH else xb
                    kk = k % KH
                    return t[:, kk * M_BLOCK + c0: kk * M_BLOCK + c0 + cw]

                # load x block as 8 x 1MiB DMAs (4 k-tiles each), alternating
                # gpsimd/scalar queues
                for kq in range(KT // 4):
                    t = xa if kq < 4 else xb
                    q0 = (kq % 4) * 4 * M_BLOCK
                    eng = nc.gpsimd if kq % 2 == 0 else nc.scalar
                    eng.dma_start(
                        t[:, q0:q0 + 4 * M_BLOCK].rearrange(
                            "p (q m) -> p q m", q=4),
                        xt[kq * 4 * P:(kq + 1) * 4 * P,
                           r0:r0 + M_BLOCK].rearrange("(q p) m -> p q m", p=P),
                    )

                # main GEMM: W fetched as adjacent k-tile pairs [128, 2*OC]
                for oc in range(N_OC):
                    psums = []
                    for mt in range(MT_PER_BLOCK):
                        psums.append(ppool.tile([P, OC], F32, tag="acc", name=f"ps_{blk}_{oc}_{mt}"))
                    for k2 in range(KT // 2):
                        wtile = wpool.tile([P, 2 * OC], BF16, tag="wt")
                        src = wt[k2 * 2 * P:(k2 + 1) * 2 * P,
                                 oc * OC:(oc + 1) * OC]
                        nc.sync.dma_start(
                            wtile.rearrange("p (b c) -> p b c", b=2),
                            src.rearrange("(b p) c -> p b c", p=P),
                        )
                        for half in range(2):
                            k = 2 * k2 + half
                            for mt in range(MT_PER_BLOCK):
                                nc.tensor.matmul(
                                    psums[mt][:],
                                    xsl(k, mt * P, P),
                                    wtile[:, half * OC:(half + 1) * OC],
                                    start=(k == 0),
                                    stop=(k == KT - 1),
                                )
                    for mt in range(MT_PER_BLOCK):
                        ot = opool.tile([P, OC], F32, tag="ot")
                        nc.vector.tensor_copy(ot[:], psums[mt][:])
                        nc.sync.dma_start(
                            out[r0 + mt * P:r0 + (mt + 1) * P,
                                oc * OC:(oc + 1) * OC],
                            ot[:],
                        )

    split_wide_waits(nc)
    return nc


_NC_CACHE = [None]


def kernel(x, weight, lora_A, lora_B):
    from concourse.bass_utils import run_bass_kernel_spmd

    x = np.asarray(x, dtype=np.float32)
    weight = np.asarray(weight, dtype=np.float32)
    lora_A = np.asarray(lora_A, dtype=np.float32)
    lora_B = np.asarray(lora_B, dtype=np.float32)

    # fold LoRA into the weight: out = x @ (W.T + 2*(B@A))
    weff = weight.T + 2.0 * (lora_B @ lora_A)
    weff_bf = np.ascontiguousarray(weff.astype(ml_dtypes.bfloat16))

    x2 = x.reshape(ROWS_TOTAL, D).astype(ml_dtypes.bfloat16)
    cz = np.zeros((P, OC), dtype=ml_dtypes.bfloat16)

    in_maps = []
    for c in range(N_CORES):
        xt_c = np.ascontiguousarray(
            x2[c * ROWS_PER_CORE:(c + 1) * ROWS_PER_CORE].T
        )
        in_maps.append({"xt": xt_c, "wt": weff_bf, "cz": cz})

    if _NC_CACHE[0] is None:
        _NC_CACHE[0] = build_program()
    nc = _NC_CACHE[0]

    res = run_bass_kernel_spmd(nc, in_maps, list(range(N_CORES)))
    out = np.concatenate(
        [res.results[c]["out"] for c in range(N_CORES)], axis=0
    )
    return out.reshape(x.shape)
